# revision 1
# baseline (speedup 1.0000x reference)
"""Trainium2 Bass kernel for nn_DBGNN (gnn_message_passing).

Math (dead first-order branch eliminated; output depends only on):
    deg  = segment_sum([ew_ho, 1s], dst+self-loops)
    dinv = rsqrt(deg)
    agg  = segment_sum(x_h[src] * (dinv[src]*ew*dinv[dst]), dst)   # A_norm @ x_h
    xh   = elu(agg @ W_ho + b_ho)
    msg  = xh @ W_bip1 + b_bip1
    bip  = segment_sum(msg[bsrc], bdst, N)
    out  = elu(bip) @ W_lin + b_lin

Sharding: destination-node blocks of N/8 per core.  Edges bucketed on host by
(core, 128-wide dst window, src half); per-window-group dma_gather of source
rows in bf16 (int16 indices -> table split in two halves); one-hot-times-norm
built in bf16 with one fused DVE tensor_scalar(is_equal, mult) per edge slot;
aggregation as PSUM-accumulated bf16 matmuls producing feature-major agg^T.

Bipartite stage routes only the needed msg rows: each producer gathers the
msg rows each consumer references (host-deduped, B rows per (p,c) pair) and
an AllToAll exchanges the 8xB blocks; consumers gather per-edge from the
received table.  This replaces an AllGather of the full msg matrix.

elu(x) = min(exp(x), max(x+1, 1)) - 1 exactly (exp(x) >= x+1 everywhere, and
for x<=0 exp(x) <= 1); the "-1" is folded into the next layer's bias.
"""
import sys

for _p in ("/opt/trn_rl_repo",):
    if _p not in sys.path:
        sys.path.append(_p)

import numpy as np

import concourse.bass as bass
import concourse.mybir as mybir
import concourse.tile as tile
from concourse import bacc
from concourse.bass_utils import run_bass_kernel_spmd

F32 = mybir.dt.float32
BF16 = mybir.dt.bfloat16
I16 = mybir.dt.int16
NPBF16 = mybir.dt.np(BF16)

NCORES = 8
F = 128      # input/hidden feature dim
H1 = 64
C = 10
W = 128      # dst window width
CHW = 4      # windows per chunk (gather batch + B/C overlap granularity)


# ---------------------------------------------------------------------------
# host-side edge bucketing
# ---------------------------------------------------------------------------

def _wrap_idx(flat):
    """dma_gather index layout: unwrapped[i] = idx16[i % 16, i // 16],
    replicated to all 8 Q7 16-partition groups."""
    t16 = flat.reshape(-1, 16).T  # [16, len/16]
    return np.tile(t16, (8, 1)).astype(np.int16)


def _bucket_half(src, core, row, wt, npc, nw, n_half):
    """Bucket edges by (core, window); `core`/`row` give each edge's
    destination core and its row (window*128+pos) within that core.
    Returns M and per-core (idx_wrapped [128, nw*M*8] int16,
    dstloc [128, nw*M] f32, norm [128, nw*M] f32)."""
    win = row >> 7
    dstloc = (row & 127).astype(np.float32)
    gwin = (core * nw + win).astype(np.int64)
    order = np.argsort(gwin, kind="stable")
    gwin_s = gwin[order]
    counts = np.bincount(gwin_s, minlength=NCORES * nw)
    M = max(1, int((counts.max() + 127) // 128))
    starts = np.zeros(NCORES * nw + 1, np.int64)
    np.cumsum(counts, out=starts[1:])
    src_s = src[order]
    dl_s = dstloc[order]
    w_s = wt[order]

    out = []
    for c in range(NCORES):
        gi = np.zeros((nw * M * 128,), np.int64)
        dl = np.zeros((nw * M * 128,), np.float32)
        nm = np.zeros((nw * M * 128,), np.float32)
        for w in range(nw):
            g = c * nw + w
            s0, s1 = starts[g], starts[g + 1]
            cnt = s1 - s0
            o = w * M * 128
            gi[o:o + cnt] = src_s[s0:s1]
            dl[o:o + cnt] = dl_s[s0:s1]
            nm[o:o + cnt] = w_s[s0:s1]
        assert gi.max(initial=0) < n_half <= 32768
        out.append((
            _wrap_idx(gi),
            np.ascontiguousarray(dl.reshape(nw * M, 128).T),
            np.ascontiguousarray(nm.reshape(nw * M, 128).T),
        ))
    return M, out


def _bucket_stream(src, core, row, wt, npc, nw):
    """Bucket edges by (core, window) for the host-pregathered stream.
    Returns M and per-core (src_flat [nw*M*128] int64 with -1 pads,
    dstloc [128, nw*M] f32, norm [128, nw*M] f32)."""
    win = row >> 7
    dstloc = (row & 127).astype(np.float32)
    gwin = (core * nw + win).astype(np.int64)
    order = np.argsort(gwin, kind="stable")
    gwin_s = gwin[order]
    counts = np.bincount(gwin_s, minlength=NCORES * nw)
    M = max(1, int((counts.max() + 127) // 128))
    starts = np.zeros(NCORES * nw + 1, np.int64)
    np.cumsum(counts, out=starts[1:])
    src_s = src[order]
    dl_s = dstloc[order]
    w_s = wt[order]

    out = []
    for c in range(NCORES):
        gi = np.full((nw * M * 128,), -1, np.int64)
        dl = np.zeros((nw * M * 128,), np.float32)
        nm = np.zeros((nw * M * 128,), np.float32)
        for w in range(nw):
            g = c * nw + w
            s0, s1 = starts[g], starts[g + 1]
            cnt = s1 - s0
            o = w * M * 128
            gi[o:o + cnt] = src_s[s0:s1]
            dl[o:o + cnt] = dl_s[s0:s1]
            nm[o:o + cnt] = w_s[s0:s1]
        out.append((
            gi,
            np.ascontiguousarray(dl.reshape(nw * M, 128).T),
            np.ascontiguousarray(nm.reshape(nw * M, 128).T),
        ))
    return M, out


def _balance(deg_lo, deg_hi, n, npc, nw, cap=None):
    """Assign destination nodes to (core, window, position) equalizing the
    max per-(window, src-half) edge count — smaller M means less gather
    padding.  Greedy LPT with per-window (128 nodes) and per-core (npc)
    capacity, then swap-refinement toward `cap` (a (cap_lo, cap_hi) pair;
    loads are compared after scaling each half by its cap)."""
    import heapq
    rl = 1.0 if cap is None else cap[1] / cap[0]
    order = np.argsort(-(deg_lo * rl + deg_hi), kind="stable")
    nwin = NCORES * nw
    filled = np.zeros(nwin, np.int64)
    ccap = np.full(NCORES, npc, np.int64)
    lo = np.zeros(nwin)
    hi = np.zeros(nwin)
    heap = [(0.0, w) for w in range(nwin)]
    heapq.heapify(heap)
    wof = np.empty(n, np.int64)       # window of node
    members = [[] for _ in range(nwin)]
    for v in order:
        while True:
            _, w = heapq.heappop(heap)
            c = w // nw
            if filled[w] < 128 and ccap[c] > 0:
                break
        wof[v] = w
        members[w].append(v)
        filled[w] += 1
        ccap[c] -= 1
        lo[w] += deg_lo[v]
        hi[w] += deg_hi[v]
        if filled[w] < 128:
            heapq.heappush(heap, (max(lo[w] * rl, hi[w]), w))

    if cap is not None:
        # swap-repair: take the most-loaded window, swap its heaviest
        # offending node with the best-fitting node of a lightly-loaded
        # window (vectorized search over candidate partners)
        dl = deg_lo.astype(np.float64) * rl
        dh = deg_hi.astype(np.float64)
        cap_s = float(cap[1])
        lo = lo * rl
        for w in range(nwin):
            members[w] = np.asarray(members[w], np.int64)
        for _ in range(3000):
            mx = np.maximum(lo, hi)
            w = int(np.argmax(mx))
            if mx[w] <= cap_s:
                break
            nodes = members[w]
            sc = (np.where(lo[w] > cap_s, dl[nodes], 0)
                  + np.where(hi[w] > cap_s, dh[nodes], 0))
            a = int(nodes[int(np.argmax(sc))])
            placed = False
            for w2 in np.argsort(mx)[:256]:
                w2 = int(w2)
                if w2 == w:
                    continue
                nodes2 = members[w2]
                nl2 = lo[w2] + dl[a] - dl[nodes2]
                nh2 = hi[w2] + dh[a] - dh[nodes2]
                nl1 = lo[w] - dl[a] + dl[nodes2]
                nh1 = hi[w] - dh[a] + dh[nodes2]
                newmx = np.maximum(np.maximum(nl2, nh2),
                                   np.maximum(nl1, nh1))
                j = int(np.argmin(newmx))
                if newmx[j] < max(mx[w], mx[w2]):
                    b = int(nodes2[j])
                    members[w] = np.concatenate(
                        [nodes[nodes != a], [b]])
                    members[w2] = np.concatenate(
                        [nodes2[nodes2 != b], [a]])
                    lo[w], hi[w] = nl1[j], nh1[j]
                    lo[w2], hi[w2] = nl2[j], nh2[j]
                    placed = True
                    break
            if not placed:
                break

    hcore = np.empty(n, np.int64)
    hrow = np.empty(n, np.int64)
    for w in range(nwin):
        for i, v in enumerate(members[w]):
            hcore[v] = w // nw
            hrow[v] = (w % nw) * 128 + i
    if cap is None:
        return hcore, hrow, float(np.maximum(lo, hi).max())
    return hcore, hrow, float(np.maximum(lo, hi).max())  # cap[1]-scaled


# ---------------------------------------------------------------------------
# Bass program
# ---------------------------------------------------------------------------

def build_nc(cfg):
    n, npc, nw = cfg["N"], cfg["NPC"], cfg["NW"]
    ma, mb = cfg["MA"], cfg["MB"]
    B = cfg["B"]
    msg_pad = nw * 128
    nbt = NCORES * B          # bipartite routed-table rows
    rep = cfg.get("REPEAT", 1)

    nc = bacc.Bacc("TRN2", target_bir_lowering=False, debug=False,
                   num_devices=NCORES, num_swdge_queues=4)

    env = {}
    e = env

    # host-pregathered per-slot source rows, stored as the SBUF image
    # [128 partitions, nw*MA slots x F] so the kernel streams them with one
    # fat contiguous descriptor per partition
    e["gx_t"] = nc.dram_tensor("gx", [128, nw * ma * F], BF16,
                               kind="ExternalInput")
    e["ad_t"] = nc.dram_tensor("a_dst", [128, nw * ma], F32,
                               kind="ExternalInput")
    e["an_t"] = nc.dram_tensor("a_nrm", [128, nw * ma], F32,
                               kind="ExternalInput")
    for nm, m in (("bip", mb),):
        e[nm + "i_t"] = nc.dram_tensor(nm + "_idx", [128, nw * m * 8], I16,
                                       kind="ExternalInput")
        e[nm + "d_t"] = nc.dram_tensor(nm + "_dst", [128, nw * m], F32,
                                       kind="ExternalInput")
        e[nm + "n_t"] = nc.dram_tensor(nm + "_nrm", [128, nw * m], F32,
                                       kind="ExternalInput")
    e["payi_t"] = nc.dram_tensor("pay_idx", [128, (NCORES * B) // 16], I16,
                                 kind="ExternalInput")
    e["xown_t"] = nc.dram_tensor("xown", [nw * 128, F], BF16,
                                 kind="ExternalInput")
    e["dinv2_t"] = nc.dram_tensor("dinv2", [128, nw], F32,
                                  kind="ExternalInput")
    e["pidx_t"] = nc.dram_tensor("pidx", [128, 1], F32,
                                 kind="ExternalInput")
    e["iota_t"] = nc.dram_tensor("iota", [128, W], BF16, kind="ExternalInput")
    e["who_t"] = nc.dram_tensor("w_ho", [F, F], BF16, kind="ExternalInput")
    e["bho_t"] = nc.dram_tensor("b_ho", [F, 1], F32, kind="ExternalInput")
    e["wbip_t"] = nc.dram_tensor("w_bip", [F, H1], BF16, kind="ExternalInput")
    e["bbip_t"] = nc.dram_tensor("b_bip", [1, H1], BF16, kind="ExternalInput")
    e["wlin_t"] = nc.dram_tensor("w_lin", [H1, C], BF16, kind="ExternalInput")
    e["blin_t"] = nc.dram_tensor("b_lin", [1, C], BF16, kind="ExternalInput")
    e["out_t"] = nc.dram_tensor("outT", [C, nw * 128], F32,
                                kind="ExternalOutput")

    with tile.TileContext(nc) as tc:
        from contextlib import ExitStack
        with ExitStack() as ctx:
            const = ctx.enter_context(tc.tile_pool(name="const", bufs=1))
            meta = ctx.enter_context(tc.tile_pool(name="meta", bufs=1))
            work = ctx.enter_context(tc.tile_pool(name="work", bufs=1))

            sb = {}
            iota_sb = const.tile([128, W], BF16)
            nc.sync.dma_start(out=iota_sb[:], in_=e["iota_t"].ap()[:, :])
            sb["iota"] = iota_sb
            for k, shape, dt in (("who", [F, F], BF16), ("bho", [F, 1], F32),
                                 ("wbip", [F, H1], BF16),
                                 ("bbip", [1, H1], BF16),
                                 ("wlin", [H1, C], BF16),
                                 ("blin", [1, C], BF16)):
                t = const.tile(shape, dt, name=k + "_sb")
                nc.sync.dma_start(out=t[:], in_=e[k + "_t"].ap()[:, :])
                sb[k] = t
            ones_sb = const.tile([1, 512], BF16)
            nc.vector.memset(ones_sb[:], 1.0)
            sb["ones"] = ones_sb
            bho1_sb = const.tile([F, 1], F32)
            nc.vector.tensor_scalar_add(out=bho1_sb[:], in0=sb["bho"][:],
                                        scalar1=1.0)
            sb["bho1"] = bho1_sb

            ad_sb = meta.tile([128, nw * ma], F32, name="ad_sb")
            nc.sync.dma_start(out=ad_sb[:], in_=e["ad_t"].ap()[:, :])
            an_sb = meta.tile([128, nw * ma], F32, name="an_sb")
            nc.sync.dma_start(out=an_sb[:], in_=e["an_t"].ap()[:, :])
            sb["astream"] = (None, ad_sb, an_sb)
            for nm, m in (("bip", mb),):
                ti = meta.tile([128, nw * m * 8], I16, name=nm + "i_sb")
                nc.sync.dma_start(out=ti[:], in_=e[nm + "i_t"].ap()[:, :])
                td = meta.tile([128, nw * m], F32, name=nm + "d_sb")
                nc.sync.dma_start(out=td[:], in_=e[nm + "d_t"].ap()[:, :])
                tn = meta.tile([128, nw * m], F32, name=nm + "n_sb")
                nc.sync.dma_start(out=tn[:], in_=e[nm + "n_t"].ap()[:, :])
                sb[nm] = (ti, td, tn)
            payi_sb = meta.tile([128, (NCORES * B) // 16], I16, name="payi_sb")
            nc.sync.dma_start(out=payi_sb[:], in_=e["payi_t"].ap()[:, :])
            sb["payi"] = payi_sb
            dinv2_sb = meta.tile([128, nw], F32, name="dinv2_sb")
            nc.sync.dma_start(out=dinv2_sb[:], in_=e["dinv2_t"].ap()[:, :])
            sb["dinv2"] = dinv2_sb
            pidx_sb = meta.tile([128, 1], F32, name="pidx_sb")
            nc.sync.dma_start(out=pidx_sb[:], in_=e["pidx_t"].ap()[:, :])
            sb["pidx"] = pidx_sb

            # cc_msg rows are fp32 (gather rows must be 256B-aligned);
            # cc_loc rows are padded to 128 bf16 for the same reason, the
            # matmul only reads the first 64 columns.
            e["cc_msg"] = nc.dram_tensor("cc_msg", [msg_pad, H1], F32,
                                         kind="Internal")
            e["cc_in"] = nc.dram_tensor("cc_in", [nbt, H1], BF16,
                                        kind="Internal")
            e["cc_out"] = nc.dram_tensor("cc_out", [nbt, H1], BF16,
                                         kind="Internal")
            # bounce into a row-padded table: gather rows must be 256B
            e["cc_loc"] = nc.dram_tensor("cc_loc", [nbt, 2 * H1], BF16,
                                         kind="Internal")

            for r in range(rep):
                _body(nc, tc, cfg, e, sb, work, r)

    nc.compile()
    return nc


def _body(nc, tc, cfg, e, sb, work, r):
    from contextlib import ExitStack
    n, npc, nw = cfg["N"], cfg["NPC"], cfg["NW"]
    ma, mb = cfg["MA"], cfg["MB"]
    B = cfg["B"]
    nbt = NCORES * B
    npay = nbt // 128        # payload slots of 128 rows

    iota_sb, ones_sb = sb["iota"], sb["ones"]
    who_sb, bho_sb, bho1_sb = sb["who"], sb["bho"], sb["bho1"]
    wbip_sb, bbip_sb = sb["wbip"], sb["bbip"]
    wlin_sb, blin_sb = sb["wlin"], sb["blin"]
    out_t = e["out_t"]
    cc_msg, cc_in, cc_out, cc_loc = (e["cc_msg"], e["cc_in"], e["cc_out"],
                                     e["cc_loc"])

    import os
    stage_lim = os.environ.get("GNN_STAGE", "full")

    with ExitStack() as stk:
        gA = stk.enter_context(tc.tile_pool(name="gA", bufs=2))
        sA = stk.enter_context(tc.tile_pool(name="sA", bufs=6))
        wA = stk.enter_context(tc.tile_pool(name="wA", bufs=2))
        psA = stk.enter_context(tc.tile_pool(name="psA", bufs=4, space="PSUM"))
        psB = stk.enter_context(tc.tile_pool(name="psB", bufs=2, space="PSUM"))
        psC = stk.enter_context(tc.tile_pool(name="psC", bufs=2, space="PSUM"))
        sB = stk.enter_context(tc.tile_pool(name="sB", bufs=3))

        # Chunks of CHW windows: the stream load for chunk k+1 overlaps
        # stages B/C of chunk k (per-chunk tiles keep dependencies narrow).
        _, dstb, nrmb = sb["astream"]
        for k in range(0, nw, CHW):
            wn = min(CHW, nw - k)
            cols = wn * 128
            # ---- stage A: agg^T[f, dst] = sum_e norm_e x_h[src_e, f] -------
            Gt = gA.tile([128, CHW * ma, F], BF16, tag="G",
                         name=f"G{r}_{k}")
            nc.sync.dma_start(
                out=Gt[:, :wn * ma, :],
                in_=e["gx_t"].ap()[:, k * ma * F:(k + wn) * ma * F])
            xo = gA.tile([128, CHW, F], BF16, tag="xo", name=f"xo{r}_{k}")
            nc.sync.dma_start(
                out=xo[:, :wn, :],
                in_=e["xown_t"].ap()[k * 128:(k + wn) * 128, :]
                .rearrange("(q p) f -> p q f", p=128))
            aggT = wA.tile([128, CHW * 128], BF16, tag="aggT",
                           name=f"aggT{r}_{k}")
            for q0 in range(0, wn, 4):
                qn = min(4, wn - q0)
                acc = psA.tile([128, 512], F32, tag="accA", space="PSUM",
                               name=f"accA{r}_{k}_{q0}")
                for wi in range(q0, q0 + qn):
                    w = k + wi
                    o = (wi - q0) * 128
                    Sd = sA.tile([128, W], BF16, tag="S", name=f"Sd{r}_{w}")
                    nc.vector.tensor_scalar(
                        out=Sd[:], in0=iota_sb[:],
                        scalar1=sb["pidx"][:, 0:1],
                        scalar2=sb["dinv2"][:, w:w + 1],
                        op0=mybir.AluOpType.is_equal,
                        op1=mybir.AluOpType.mult)
                    nc.tensor.matmul(out=acc[:, o:o + 128],
                                     lhsT=xo[:, wi, :],
                                     rhs=Sd[:], start=True, stop=False)
                    for t in range(ma):
                        col = w * ma + t
                        S = sA.tile([128, W], BF16, tag="S",
                                    name=f"S{r}_{w}_{t}")
                        nc.vector.tensor_scalar(
                            out=S[:], in0=iota_sb[:],
                            scalar1=dstb[:, col:col + 1],
                            scalar2=nrmb[:, col:col + 1],
                            op0=mybir.AluOpType.is_equal,
                            op1=mybir.AluOpType.mult)
                        nc.tensor.matmul(out=acc[:, o:o + 128],
                                         lhsT=Gt[:, wi * ma + t, :],
                                         rhs=S[:], start=False,
                                         stop=(t == ma - 1))
                nc.scalar.copy(out=aggT[:, q0 * 128:q0 * 128 + qn * 128],
                               in_=acc[:, :qn * 128])

            # ---- stage B: xh' = min(exp(z), z+1), z = W_ho^T aggT + b ------
            xhT = wA.tile([128, CHW * 128], BF16, tag="xhT",
                          name=f"xhT{r}_{k}")
            for j in range((cols + 511) // 512):
                nt = min(512, cols - j * 512)
                zB = psB.tile([128, 512], F32, tag="zB", name=f"zB{r}_{k}_{j}",
                              space="PSUM")
                nc.tensor.matmul(out=zB[:, :nt], lhsT=who_sb[:],
                                 rhs=aggT[:, j * 512:j * 512 + nt],
                                 start=True, stop=True)
                eB = sB.tile([128, 512], BF16, tag="eB", name=f"eB{r}_{k}_{j}")
                nc.scalar.activation(out=eB[:, :nt], in_=zB[:, :nt],
                                     func=mybir.ActivationFunctionType.Exp,
                                     bias=bho_sb[:], scale=1.0)
                zbB = sB.tile([128, 512], BF16, tag="zbB",
                              name=f"zbB{r}_{k}_{j}")
                nc.vector.tensor_scalar(out=zbB[:, :nt], in0=zB[:, :nt],
                                        scalar1=bho1_sb[:], scalar2=1.0,
                                        op0=mybir.AluOpType.add,
                                        op1=mybir.AluOpType.max)
                nc.vector.tensor_tensor(out=xhT[:, j * 512:j * 512 + nt],
                                        in0=eB[:, :nt], in1=zbB[:, :nt],
                                        op=mybir.AluOpType.min)

            if stage_lim == "A":
                oX = sB.tile([C, CHW * 128], F32, tag="oX", name=f"oX{r}_{k}")
                nc.vector.tensor_copy(out=oX[:, :cols], in_=xhT[:C, :cols])
                nc.sync.dma_start(
                    out=out_t.ap()[:, k * 128:k * 128 + cols],
                    in_=oX[:, :cols])
                continue

            # ---- stage C: msg = xh' W_bip + b'  (node-major, fp32) ---------
            for q0 in range(0, wn, 4):
                qn = min(4, wn - q0)
                zC = psC.tile([128, 4, H1], F32, tag="zC",
                              name=f"zC{r}_{k}_{q0}", space="PSUM")
                for qi in range(qn):
                    wi = q0 + qi
                    nc.tensor.matmul(out=zC[:, qi, :],
                                     lhsT=xhT[:, wi * 128:(wi + 1) * 128],
                                     rhs=wbip_sb[:], start=True, stop=False)
                    nc.tensor.matmul(out=zC[:, qi, :], lhsT=ones_sb[:, :128],
                                     rhs=bbip_sb[:], start=False, stop=True)
                oC = sB.tile([128, 4, H1], F32, tag="oC",
                             name=f"oC{r}_{k}_{q0}")
                nc.scalar.copy(out=oC[:, :qn, :], in_=zC[:, :qn, :])
                nc.sync.dma_start(
                    out=cc_msg.ap()[(k + q0) * 128:(k + q0 + qn) * 128, :]
                    .rearrange("(q p) f -> p q f", p=128),
                    in_=oC[:, :qn, :])

        if stage_lim == "A":
            return

        # ---- payload: rows each consumer needs, in its slot order ----------
        pay32 = sB.tile([128, npay, H1], F32, tag="pay32", name=f"pay32{r}")
        nc.gpsimd.dma_gather(
            out_ap=pay32[:, :, :],
            in_ap=cc_msg.ap()[:, :],
            idxs_ap=sb["payi"][:, :],
            num_idxs=nbt, num_idxs_reg=nbt,
            elem_size=H1, single_packet=False, queue_num=1)
        pay = sB.tile([128, npay, H1], BF16, tag="pay", name=f"pay{r}")
        # on Act: keeps the in-order DVE stream free to pre-build stage-D
        # one-hot tiles while the payload gather + exchange are in flight
        nc.scalar.copy(out=pay[:, :, :], in_=pay32[:, :, :])
        nc.sync.dma_start(
            out=cc_in.ap().rearrange("(s p) f -> p s f", p=128),
            in_=pay[:, :, :])

        if os.environ.get("GNN_NOCC", "0") == "1":
            # timing-only variant: skip the exchange (results are wrong)
            nc.sync.dma_start(out=cc_loc.ap()[:, :H1], in_=cc_in.ap()[:, :])
        else:
            nc.gpsimd.collective_compute(
                kind="AllToAll", op=mybir.AluOpType.bypass,
                replica_groups=[list(range(NCORES))],
                ins=[cc_in.ap()[:, :]], outs=[cc_out.ap()[:, :]])
            nc.sync.dma_start(out=cc_loc.ap()[:, :H1],
                              in_=cc_out.ap()[:, :])

        if stage_lim == "C":
            return

    # ============ stage D: bip' = exp-min of bipartite scatter ===============
    with ExitStack() as stk2:
        gD = stk2.enter_context(tc.tile_pool(name="gD", bufs=3))
        # Sb ring sized to hold every bipartite one-hot tile: DVE builds them
        # all during the collective, so post-exchange only matmul/exp remain.
        sD = stk2.enter_context(tc.tile_pool(name="sD", bufs=2 * ((nw * mb)
                                                                 // 2 + 4)))
        eD_pool = stk2.enter_context(tc.tile_pool(name="eDp", bufs=6))
        wD = stk2.enter_context(tc.tile_pool(name="wD", bufs=2))
        psD = stk2.enter_context(tc.tile_pool(name="psD", bufs=6, space="PSUM"))
        psF = stk2.enter_context(tc.tile_pool(name="psF", bufs=2, space="PSUM"))
        sF = stk2.enter_context(tc.tile_pool(name="sF", bufs=3))

        idx_sb, dstb, nrmb = sb["bip"]
        for ki, k in enumerate(range(0, nw, CHW)):
            wn = min(CHW, nw - k)
            Gt = gD.tile([128, CHW * mb, 2 * H1], BF16, tag="Gbip",
                         name=f"Gbip_{r}_{k}")
            nc.gpsimd.dma_gather(
                out_ap=Gt[:, :wn * mb, :],
                in_ap=cc_loc.ap()[:, :],
                idxs_ap=idx_sb[:, k * mb * 8:(k + wn) * mb * 8],
                num_idxs=wn * mb * 128,
                num_idxs_reg=wn * mb * 128,
                elem_size=2 * H1, single_packet=False,
                queue_num=ki % 4)
            bipT = wD.tile([H1, CHW * 128], BF16, tag="bipT",
                           name=f"bipT{r}_{k}")
            for wi in range(wn):
                w = k + wi
                accD = psD.tile([H1, W], F32, tag="accD", space="PSUM",
                                name=f"accD{r}_{w}")
                for t in range(mb):
                    col = w * mb + t
                    Sb = sD.tile([128, W], BF16, tag="Sb",
                                 name=f"Sb{r}_{w}_{t}")
                    nc.vector.tensor_scalar(
                        out=Sb[:], in0=iota_sb[:],
                        scalar1=dstb[:, col:col + 1],
                        scalar2=nrmb[:, col:col + 1],
                        op0=mybir.AluOpType.is_equal,
                        op1=mybir.AluOpType.mult)
                    nc.tensor.matmul(out=accD[:],
                                     lhsT=Gt[:, wi * mb + t, :H1],
                                     rhs=Sb[:], start=(t == 0),
                                     stop=(t == mb - 1))
                eD = eD_pool.tile([H1, W], BF16, tag="eD", name=f"eD{r}_{w}")
                nc.scalar.activation(out=eD[:], in_=accD[:],
                                     func=mybir.ActivationFunctionType.Exp)
                zbD = eD_pool.tile([H1, W], BF16, tag="zbD",
                                   name=f"zbD{r}_{w}")
                nc.vector.tensor_scalar(out=zbD[:], in0=accD[:],
                                        scalar1=1.0, scalar2=1.0,
                                        op0=mybir.AluOpType.add,
                                        op1=mybir.AluOpType.max)
                nc.vector.tensor_tensor(out=bipT[:, wi * 128:(wi + 1) * 128],
                                        in0=eD[:], in1=zbD[:],
                                        op=mybir.AluOpType.min)

            # ---- stage F: out^T = W_lin'^T bip' + b'' ----------------------
            fcols = wn * 128
            for j in range((fcols + 511) // 512):
                nt = min(512, fcols - j * 512)
                zF = psF.tile([C, 512], F32, tag="zF", name=f"zF{r}_{k}_{j}",
                              space="PSUM")
                nc.tensor.matmul(out=zF[:, :nt], lhsT=wlin_sb[:],
                                 rhs=bipT[:, j * 512:j * 512 + nt],
                                 start=True, stop=False)
                nc.tensor.matmul(out=zF[:, :nt], lhsT=blin_sb[:],
                                 rhs=ones_sb[:, :nt], start=False, stop=True)
                oF = sF.tile([C, 512], F32, tag="oF", name=f"oF{r}_{k}_{j}")
                nc.vector.tensor_copy(out=oF[:, :nt], in_=zF[:, :nt])
                nc.sync.dma_start(
                    out=out_t.ap()[:, k * 128 + j * 512:
                                   k * 128 + j * 512 + nt],
                    in_=oF[:, :nt])


# ---------------------------------------------------------------------------
# public entry
# ---------------------------------------------------------------------------

def _prepare(inputs, n):
    npc = n // NCORES
    nw = (npc + 127) // 128

    ei = np.asarray(inputs["edge_index_higher_order"])
    src = ei[0].astype(np.int64)
    dst = ei[1].astype(np.int64)
    ew = np.asarray(inputs["edge_weights_higher_order"]).astype(np.float64)

    deg = np.bincount(dst, weights=ew, minlength=n) + 1.0
    dinv = 1.0 / np.sqrt(deg)
    norm = (dinv[src] * ew * dinv[dst]).astype(np.float32)

    # balance higher-order nodes over (core, window) by in-degree excluding
    # self-loops (those are computed from the contiguous own-block table):
    # near-perfect balance puts every window at M=16 slots of 128 edges.
    dcnt = np.bincount(dst, minlength=n)
    hcore, hrow, _ = _balance(dcnt, np.zeros(n, np.int64), n, npc, nw,
                              cap=(2048, 2048))

    ma, bkt_a = _bucket_stream(src, hcore[dst], hrow[dst], norm, npc, nw)

    # ---- bipartite routing: dedup (producer, consumer) rows, fixed block B
    bi = np.asarray(inputs["bipartite_edge_index"])
    bsrc = bi[0].astype(np.int64)
    bdst = bi[1].astype(np.int64)

    # balance first-order (output) nodes by bipartite in-degree: M_b=1 if
    # every window stays <= 128 edges
    bdeg = np.bincount(bdst, minlength=n)
    ocore, orow, omx = _balance(bdeg, np.zeros(n, np.int64), n, npc, nw,
                                cap=(128, 128))
    if omx > 128:
        ocore = np.arange(n) // npc
        orow = np.arange(n) - ocore * npc

    ncons = ocore[bdst]
    nprod = hcore[bsrc]
    srow = hrow[bsrc]          # producer-local msg row of each edge's source
    maxu = 0
    for c in range(NCORES):
        for p in range(NCORES):
            m = (ncons == c) & (nprod == p)
            maxu = max(maxu, len(np.unique(srow[m])))
    B = ((maxu + 127) // 128) * 128

    pay_idx = np.zeros((NCORES, NCORES * B), np.int64)
    table_row = np.zeros(len(bsrc), np.int64)
    for c in range(NCORES):
        cm = ncons == c
        for p in range(NCORES):
            m = cm & (nprod == p)
            uniq, inv = np.unique(srow[m], return_inverse=True)
            pay_idx[p, c * B:c * B + len(uniq)] = uniq
            table_row[m] = p * B + inv

    mb, bkt_b = _bucket_half(table_row, ncons, orow[bdst],
                             np.ones(len(bsrc), np.float32),
                             npc, nw, NCORES * B)

    cfg = dict(N=n, NPC=npc, NW=nw, B=B, MA=ma, MB=mb)
    buckets = dict(astream=bkt_a, bip=bkt_b, pay=pay_idx,
                   hcore=hcore, hrow=hrow, ocore=ocore, orow=orow,
                   dinv=dinv)
    return cfg, buckets


def make_in_maps(inputs, cfg, buckets):
    n = cfg["N"]
    ma = cfg["MA"]
    x_h = np.asarray(inputs["x_h"], dtype=np.float32).astype(NPBF16)
    x_h = np.ascontiguousarray(x_h)

    W_ho = np.asarray(inputs["W_ho"], np.float32)
    b_ho = np.asarray(inputs["b_ho"], np.float32)
    W_bip = np.asarray(inputs["W_bip1"], np.float32)
    b_bip = np.asarray(inputs["b_bip1"], np.float32)
    W_lin = np.asarray(inputs["W_lin"], np.float32)
    b_lin = np.asarray(inputs["b_lin"], np.float32)

    b_bip_eff = (b_bip - W_bip.sum(axis=0)).reshape(1, H1)
    b_lin_eff = (b_lin - W_lin.sum(axis=0)).reshape(1, C)
    iota = np.broadcast_to(np.arange(W, dtype=np.float32),
                           (128, W)).astype(NPBF16).copy()

    nw = cfg["NW"]
    hcore, hrow = buckets["hcore"], buckets["hrow"]
    dinv = buckets["dinv"]
    in_maps = []
    for c in range(NCORES):
        src_flat, adst, anrm = buckets["astream"][c]
        gxr = np.zeros((nw * ma * 128, F), NPBF16)
        emask = src_flat >= 0
        gxr[emask] = x_h[src_flat[emask]]
        gx = np.ascontiguousarray(
            gxr.reshape(nw * ma, 128, F).transpose(1, 0, 2)
            .reshape(128, nw * ma * F))
        m = {
            "gx": gx,
            "a_dst": adst,
            "a_nrm": anrm,
            "iota": iota,
            "w_ho": np.ascontiguousarray(W_ho).astype(NPBF16),
            "b_ho": b_ho.reshape(F, 1).astype(np.float32),
            "w_bip": np.ascontiguousarray(W_bip).astype(NPBF16),
            "b_bip": b_bip_eff.astype(NPBF16),
            "w_lin": np.ascontiguousarray(W_lin).astype(NPBF16),
            "b_lin": b_lin_eff.astype(NPBF16),
            "pay_idx": _wrap_idx(buckets["pay"][c]),
        }
        mask = hcore == c
        rows = hrow[mask]
        xown = np.zeros((nw * 128, F), NPBF16)
        xown[rows] = x_h[mask]
        dv2 = np.zeros((128, nw), np.float32)
        dv2[rows % 128, rows // 128] = (dinv[mask] ** 2)
        m["xown"] = xown
        m["dinv2"] = dv2
        m["pidx"] = np.arange(128, dtype=np.float32).reshape(128, 1)
        gi, dl, nr = buckets["bip"][c]
        m["bip_idx"] = gi
        m["bip_dst"] = dl
        m["bip_nrm"] = nr
        in_maps.append(m)
    return in_maps


def kernel(**inputs):
    x_h = np.asarray(inputs["x_h"])
    n = x_h.shape[0]
    cfg, buckets = _prepare(inputs, n)
    nc = build_nc(cfg)
    in_maps = make_in_maps(inputs, cfg, buckets)
    res = run_bass_kernel_spmd(nc, in_maps, core_ids=list(range(NCORES)))
    arr = np.stack([res.results[c]["outT"] for c in range(NCORES)])
    return np.ascontiguousarray(
        arr[buckets["ocore"], :, buckets["orow"]]).astype(np.float32)



# revision 3
# speedup vs baseline: 1.3632x; 1.3632x over previous
"""Trainium2 Bass kernel for nn_DBGNN (gnn_message_passing).

Math (dead first-order branch eliminated; output depends only on):
    deg  = segment_sum([ew_ho, 1s], dst+self-loops)          (over ALL edges)
    dinv = rsqrt(deg)
    agg  = segment_sum(x_h[src] * (dinv[src]*ew*dinv[dst]), dst)   # A_norm @ x_h
    xh   = elu(agg @ W_ho + b_ho)
    msg  = xh @ W_bip1 + b_bip1
    bip  = segment_sum(msg[bsrc], bdst, N)
    out  = elu(bip) @ W_lin + b_lin

Dead-node pruning: only higher-order nodes referenced by a bipartite edge
(~63%) contribute to the output; stage A/B/C run on those alone.  Self-loops
are folded into the edge stream as ordinary edges with norm=dinv^2.

Sharding: destination-node blocks per core.  Edges bucketed on host by
(core, 128-wide dst window); host-pregathered bf16 source rows stream per
chunk; one-hot-times-norm built with one fused DVE tensor_scalar per edge
slot; aggregation as PSUM-accumulated bf16 matmuls producing feature-major
agg^T.

Bipartite stage routes only the needed msg rows: stage C writes msg into a
256B-row bf16 table (cols 64..127 junk, never read); each producer gathers
the rows each consumer references (host-deduped, B rows per (p,c) pair); an
AllToAll exchanges the 8xB blocks; the received table is gathered directly
by consumers (256B rows, no re-pad bounce).

elu(x) = min(exp(x), max(x+1, 1)) - 1 exactly (exp(x) >= x+1 everywhere, and
for x<=0 exp(x) <= 1); the "-1" is folded into the next layer's bias.
"""
import sys

for _p in ("/opt/trn_rl_repo",):
    if _p not in sys.path:
        sys.path.append(_p)

import numpy as np

import concourse.bass as bass
import concourse.mybir as mybir
import concourse.tile as tile
from concourse import bacc
from concourse.bass_utils import run_bass_kernel_spmd

F32 = mybir.dt.float32
BF16 = mybir.dt.bfloat16
I16 = mybir.dt.int16
NPBF16 = mybir.dt.np(BF16)

NCORES = 8
F = 128      # input/hidden feature dim
H1 = 64
C = 10
W = 128      # dst window width
CHW = 4      # windows per chunk (stream batch granularity)


# ---------------------------------------------------------------------------
# host-side edge bucketing
# ---------------------------------------------------------------------------

def _wrap_idx(flat):
    """dma_gather index layout: unwrapped[i] = idx16[i % 16, i // 16],
    replicated to all 8 Q7 16-partition groups."""
    t16 = flat.reshape(-1, 16).T  # [16, len/16]
    return np.tile(t16, (8, 1)).astype(np.int16)


def _bucket_edges(src, core, row, wt, nw, pad_idx):
    """Bucket edges by (core, window); `core`/`row` give each edge's
    destination core and its row (window*128+pos) within that core.
    Returns M and per-core (src_flat [nw*M*128] int64 with pad_idx pads,
    dstloc [128, nw*M] f32, norm [128, nw*M] f32)."""
    win = row >> 7
    dstloc = (row & 127).astype(np.float32)
    gwin = (core * nw + win).astype(np.int64)
    order = np.argsort(gwin, kind="stable")
    gwin_s = gwin[order]
    counts = np.bincount(gwin_s, minlength=NCORES * nw)
    M = max(1, int((counts.max() + 127) // 128))
    starts = np.zeros(NCORES * nw + 1, np.int64)
    np.cumsum(counts, out=starts[1:])
    src_s = src[order]
    dl_s = dstloc[order]
    w_s = wt[order]

    out = []
    for c in range(NCORES):
        gi = np.full((nw * M * 128,), pad_idx, np.int64)
        dl = np.zeros((nw * M * 128,), np.float32)
        nm = np.zeros((nw * M * 128,), np.float32)
        for w in range(nw):
            g = c * nw + w
            s0, s1 = starts[g], starts[g + 1]
            cnt = s1 - s0
            o = w * M * 128
            gi[o:o + cnt] = src_s[s0:s1]
            dl[o:o + cnt] = dl_s[s0:s1]
            nm[o:o + cnt] = w_s[s0:s1]
        out.append((
            gi,
            np.ascontiguousarray(dl.reshape(nw * M, 128).T),
            np.ascontiguousarray(nm.reshape(nw * M, 128).T),
        ))
    return M, out


def _balance(nodes, deg, nwin, cap, core_cap=None, ncores=NCORES):
    """Assign `nodes` (weights `deg`) to ncores*nwin windows of 128
    positions, minimizing the max per-window weight.  Greedy LPT with
    per-window (128 nodes) and optional per-core position capacity, then
    swap-refinement toward `cap`.  Returns (win_of, pos_of, maxload)."""
    import heapq
    order = np.argsort(-deg, kind="stable")
    nwin_t = ncores * nwin
    filled = np.zeros(nwin_t, np.int64)
    ccap = np.full(ncores, core_cap if core_cap else nwin * 128, np.int64)
    load = np.zeros(nwin_t)
    heap = [(0.0, w) for w in range(nwin_t)]
    heapq.heapify(heap)
    members = [[] for _ in range(nwin_t)]
    win_of = np.empty(len(nodes), np.int64)
    for i in order:
        while True:
            _, w = heapq.heappop(heap)
            c = w // nwin
            if filled[w] < 128 and ccap[c] > 0:
                break
        win_of[i] = w
        members[w].append(i)
        filled[w] += 1
        ccap[c] -= 1
        load[w] += deg[i]
        if filled[w] < 128:
            heapq.heappush(heap, (load[w], w))

    dl = deg.astype(np.float64)
    for w in range(nwin_t):
        members[w] = np.asarray(members[w], np.int64)
    for _ in range(4000):
        w = int(np.argmax(load))
        if load[w] <= cap:
            break
        nodes_w = members[w]
        a_i = int(nodes_w[int(np.argmax(dl[nodes_w]))])
        placed = False
        for w2 in np.argsort(load)[:256]:
            w2 = int(w2)
            if w2 == w or (w2 // nwin) != (w // nwin) and False:
                continue
            if w2 == w:
                continue
            nodes2 = members[w2]
            if len(nodes2) == 0:
                continue
            nl2 = load[w2] + dl[a_i] - dl[nodes2]
            nl1 = load[w] - dl[a_i] + dl[nodes2]
            newmx = np.maximum(nl2, nl1)
            j = int(np.argmin(newmx))
            if newmx[j] < max(load[w], load[w2]):
                b_i = int(nodes2[j])
                members[w] = np.concatenate([nodes_w[nodes_w != a_i], [b_i]])
                members[w2] = np.concatenate([nodes2[nodes2 != b_i], [a_i]])
                load[w], load[w2] = nl1[j], nl2[j]
                placed = True
                break
        if not placed:
            break

    pos_of = np.empty(len(nodes), np.int64)
    for w in range(nwin_t):
        for p, i in enumerate(members[w]):
            win_of[i] = w
            pos_of[i] = p
    return win_of, pos_of, float(load.max())


# ---------------------------------------------------------------------------
# Bass program
# ---------------------------------------------------------------------------

def build_nc(cfg):
    nwa, nwd = cfg["NWA"], cfg["NWD"]
    ma, mb = cfg["MA"], cfg["MB"]
    B = cfg["B"]
    nbt = NCORES * B          # bipartite routed-table rows
    rep = cfg.get("REPEAT", 1)

    nc = bacc.Bacc("TRN2", target_bir_lowering=False, debug=False,
                   num_devices=NCORES, num_swdge_queues=4)

    env = {}
    e = env

    # host-pregathered per-slot source rows, stored as the SBUF image
    # [128 partitions, nwa*MA slots x F] so the kernel streams them with one
    # fat contiguous descriptor per partition
    e["gx_t"] = nc.dram_tensor("gx", [128, nwa * ma * F], BF16,
                               kind="ExternalInput")
    e["ad_t"] = nc.dram_tensor("a_dst", [128, nwa * ma], F32,
                               kind="ExternalInput")
    e["an_t"] = nc.dram_tensor("a_nrm", [128, nwa * ma], F32,
                               kind="ExternalInput")
    e["bipi_t"] = nc.dram_tensor("bip_idx", [128, nwd * mb * 8], I16,
                                 kind="ExternalInput")
    e["bipd_t"] = nc.dram_tensor("bip_dst", [128, nwd * mb], F32,
                                 kind="ExternalInput")
    e["bipn_t"] = nc.dram_tensor("bip_nrm", [128, nwd * mb], F32,
                                 kind="ExternalInput")
    e["payi_t"] = nc.dram_tensor("pay_idx", [128, (NCORES * B) // 16], I16,
                                 kind="ExternalInput")
    e["iota_t"] = nc.dram_tensor("iota", [128, W], BF16, kind="ExternalInput")
    e["who_t"] = nc.dram_tensor("w_ho", [F, F], BF16, kind="ExternalInput")
    e["bho_t"] = nc.dram_tensor("b_ho", [F, 1], F32, kind="ExternalInput")
    e["wbip_t"] = nc.dram_tensor("w_bip", [F, H1], BF16, kind="ExternalInput")
    e["bbip_t"] = nc.dram_tensor("b_bip", [1, H1], BF16, kind="ExternalInput")
    e["wlin_t"] = nc.dram_tensor("w_lin", [H1, C], BF16, kind="ExternalInput")
    e["blin_t"] = nc.dram_tensor("b_lin", [1, C], BF16, kind="ExternalInput")
    e["out_t"] = nc.dram_tensor("outT", [C, nwd * 128], F32,
                                kind="ExternalOutput")

    with tile.TileContext(nc) as tc:
        from contextlib import ExitStack
        with ExitStack() as ctx:
            const = ctx.enter_context(tc.tile_pool(name="const", bufs=1))
            meta = ctx.enter_context(tc.tile_pool(name="meta", bufs=1))
            work = ctx.enter_context(tc.tile_pool(name="work", bufs=1))

            sb = {}
            iota_sb = const.tile([128, W], BF16)
            nc.sync.dma_start(out=iota_sb[:], in_=e["iota_t"].ap()[:, :])
            sb["iota"] = iota_sb
            for k, shape, dt in (("who", [F, F], BF16), ("bho", [F, 1], F32),
                                 ("wbip", [F, H1], BF16),
                                 ("bbip", [1, H1], BF16),
                                 ("wlin", [H1, C], BF16),
                                 ("blin", [1, C], BF16)):
                t = const.tile(shape, dt, name=k + "_sb")
                nc.sync.dma_start(out=t[:], in_=e[k + "_t"].ap()[:, :])
                sb[k] = t
            ones_sb = const.tile([1, 512], BF16)
            nc.vector.memset(ones_sb[:], 1.0)
            sb["ones"] = ones_sb
            bho1_sb = const.tile([F, 1], F32)
            nc.vector.tensor_scalar_add(out=bho1_sb[:], in0=sb["bho"][:],
                                        scalar1=1.0)
            sb["bho1"] = bho1_sb

            ad_sb = meta.tile([128, nwa * ma], F32, name="ad_sb")
            nc.sync.dma_start(out=ad_sb[:], in_=e["ad_t"].ap()[:, :])
            an_sb = meta.tile([128, nwa * ma], F32, name="an_sb")
            nc.sync.dma_start(out=an_sb[:], in_=e["an_t"].ap()[:, :])
            sb["astream"] = (ad_sb, an_sb)
            ti = meta.tile([128, nwd * mb * 8], I16, name="bipi_sb")
            nc.sync.dma_start(out=ti[:], in_=e["bipi_t"].ap()[:, :])
            td = meta.tile([128, nwd * mb], F32, name="bipd_sb")
            nc.sync.dma_start(out=td[:], in_=e["bipd_t"].ap()[:, :])
            tn = meta.tile([128, nwd * mb], F32, name="bipn_sb")
            nc.sync.dma_start(out=tn[:], in_=e["bipn_t"].ap()[:, :])
            sb["bip"] = (ti, td, tn)
            payi_sb = meta.tile([128, (NCORES * B) // 16], I16, name="payi_sb")
            nc.sync.dma_start(out=payi_sb[:], in_=e["payi_t"].ap()[:, :])
            sb["payi"] = payi_sb

            # msg table: 256B rows (bf16 x128), cols 64..127 junk/never read
            e["cc_msg"] = nc.dram_tensor("cc_msg", [nwa * 128, 2 * H1], BF16,
                                         kind="Internal")
            e["cc_in"] = nc.dram_tensor("cc_in", [nbt, 2 * H1], BF16,
                                        kind="Internal")
            e["cc_out"] = nc.dram_tensor("cc_out", [nbt, 2 * H1], BF16,
                                         kind="Internal")

            for r in range(rep):
                _body(nc, tc, cfg, e, sb, work, r)

    nc.compile()
    return nc


def _body(nc, tc, cfg, e, sb, work, r):
    from contextlib import ExitStack
    nwa, nwd = cfg["NWA"], cfg["NWD"]
    ma, mb = cfg["MA"], cfg["MB"]
    B = cfg["B"]
    nbt = NCORES * B
    npay = nbt // 128        # payload slots of 128 rows

    iota_sb, ones_sb = sb["iota"], sb["ones"]
    who_sb, bho_sb, bho1_sb = sb["who"], sb["bho"], sb["bho1"]
    wbip_sb, bbip_sb = sb["wbip"], sb["bbip"]
    wlin_sb, blin_sb = sb["wlin"], sb["blin"]
    out_t = e["out_t"]
    cc_msg, cc_in, cc_out = e["cc_msg"], e["cc_in"], e["cc_out"]

    import os
    stage_lim = os.environ.get("GNN_STAGE", "full")

    with ExitStack() as stk:
        gA = stk.enter_context(tc.tile_pool(name="gA", bufs=2))
        sA = stk.enter_context(tc.tile_pool(name="sA", bufs=6))
        wA = stk.enter_context(tc.tile_pool(name="wA", bufs=2))
        psA = stk.enter_context(tc.tile_pool(name="psA", bufs=4, space="PSUM"))
        psB = stk.enter_context(tc.tile_pool(name="psB", bufs=2, space="PSUM"))
        psC = stk.enter_context(tc.tile_pool(name="psC", bufs=2, space="PSUM"))
        sB = stk.enter_context(tc.tile_pool(name="sB", bufs=3))

        # Chunks of CHW windows: the stream load for chunk k+1 overlaps
        # stages B/C of chunk k (per-chunk tiles keep dependencies narrow).
        dstb, nrmb = sb["astream"]
        for k in range(0, nwa, CHW):
            wn = min(CHW, nwa - k)
            cols = wn * 128
            # ---- stage A: agg^T[f, dst] = sum_e norm_e x_h[src_e, f] -------
            Gt = gA.tile([128, CHW * ma, F], BF16, tag="G",
                         name=f"G{r}_{k}")
            nc.sync.dma_start(
                out=Gt[:, :wn * ma, :],
                in_=e["gx_t"].ap()[:, k * ma * F:(k + wn) * ma * F])
            aggT = wA.tile([128, CHW * 128], BF16, tag="aggT",
                           name=f"aggT{r}_{k}")
            acc = psA.tile([128, 512], F32, tag="accA", space="PSUM",
                           name=f"accA{r}_{k}")
            for wi in range(wn):
                w = k + wi
                o = wi * 128
                for t in range(ma):
                    col = w * ma + t
                    S = sA.tile([128, W], BF16, tag="S",
                                name=f"S{r}_{w}_{t}")
                    nc.vector.tensor_scalar(
                        out=S[:], in0=iota_sb[:],
                        scalar1=dstb[:, col:col + 1],
                        scalar2=nrmb[:, col:col + 1],
                        op0=mybir.AluOpType.is_equal,
                        op1=mybir.AluOpType.mult)
                    nc.tensor.matmul(out=acc[:, o:o + 128],
                                     lhsT=Gt[:, wi * ma + t, :],
                                     rhs=S[:], start=(t == 0),
                                     stop=(t == ma - 1))
            nc.scalar.copy(out=aggT[:, :cols], in_=acc[:, :cols])

            # ---- stage B: xh' = min(exp(z), z+1), z = W_ho^T aggT + b ------
            xhT = wA.tile([128, CHW * 128], BF16, tag="xhT",
                          name=f"xhT{r}_{k}")
            zB = psB.tile([128, 512], F32, tag="zB", name=f"zB{r}_{k}",
                          space="PSUM")
            nc.tensor.matmul(out=zB[:, :cols], lhsT=who_sb[:],
                             rhs=aggT[:, :cols],
                             start=True, stop=True)
            eB = sB.tile([128, 512], BF16, tag="eB", name=f"eB{r}_{k}")
            nc.scalar.activation(out=eB[:, :cols], in_=zB[:, :cols],
                                 func=mybir.ActivationFunctionType.Exp,
                                 bias=bho_sb[:], scale=1.0)
            zbB = sB.tile([128, 512], BF16, tag="zbB",
                          name=f"zbB{r}_{k}")
            nc.vector.tensor_scalar(out=zbB[:, :cols], in0=zB[:, :cols],
                                    scalar1=bho1_sb[:], scalar2=1.0,
                                    op0=mybir.AluOpType.add,
                                    op1=mybir.AluOpType.max)
            nc.vector.tensor_tensor(out=xhT[:, :cols],
                                    in0=eB[:, :cols], in1=zbB[:, :cols],
                                    op=mybir.AluOpType.min)

            if stage_lim == "A":
                oX = sB.tile([C, CHW * 128], F32, tag="oX", name=f"oX{r}_{k}")
                nc.vector.tensor_copy(out=oX[:, :cols], in_=xhT[:C, :cols])
                nc.sync.dma_start(
                    out=out_t.ap()[:, :cols] if k == 0 else
                    out_t.ap()[:, :cols],
                    in_=oX[:, :cols])
                continue

            # ---- stage C: msg = xh' W_bip + b'  (node-major, bf16) ---------
            zC = psC.tile([128, CHW, H1], F32, tag="zC",
                          name=f"zC{r}_{k}", space="PSUM")
            for wi in range(wn):
                nc.tensor.matmul(out=zC[:, wi, :],
                                 lhsT=xhT[:, wi * 128:(wi + 1) * 128],
                                 rhs=wbip_sb[:], start=True, stop=False)
                nc.tensor.matmul(out=zC[:, wi, :], lhsT=ones_sb[:, :128],
                                 rhs=bbip_sb[:], start=False, stop=True)
            oC = sB.tile([128, CHW, H1], BF16, tag="oC",
                         name=f"oC{r}_{k}")
            nc.scalar.copy(out=oC[:, :wn, :], in_=zC[:, :wn, :])
            nc.sync.dma_start(
                out=cc_msg.ap()[k * 128:(k + wn) * 128, :H1]
                .rearrange("(q p) f -> p q f", p=128),
                in_=oC[:, :wn, :])

        if stage_lim == "A":
            return

        # ---- payload: rows each consumer needs, in its slot order ----------
        pay = sB.tile([128, npay, 2 * H1], BF16, tag="pay", name=f"pay{r}")
        nc.gpsimd.dma_gather(
            out_ap=pay[:, :, :],
            in_ap=cc_msg.ap()[:, :],
            idxs_ap=sb["payi"][:, :],
            num_idxs=nbt, num_idxs_reg=nbt,
            elem_size=2 * H1, single_packet=False, queue_num=1)
        nc.sync.dma_start(
            out=cc_in.ap().rearrange("(s p) f -> p s f", p=128),
            in_=pay[:, :, :])

        if os.environ.get("GNN_NOCC", "0") == "1":
            # timing-only variant: skip the exchange (results are wrong)
            nc.sync.dma_start(out=cc_out.ap()[:, :], in_=cc_in.ap()[:, :])
        else:
            nc.gpsimd.collective_compute(
                kind="AllToAll", op=mybir.AluOpType.bypass,
                replica_groups=[list(range(NCORES))],
                ins=[cc_in.ap()[:, :]], outs=[cc_out.ap()[:, :]])

        if stage_lim == "C":
            return

    # ============ stage D: bip' = exp-min of bipartite scatter ===============
    with ExitStack() as stk2:
        gD = stk2.enter_context(tc.tile_pool(name="gD", bufs=4))
        # Sb ring sized to hold every bipartite one-hot tile: DVE builds them
        # all during the collective, so post-exchange only matmul/exp remain.
        sD = stk2.enter_context(tc.tile_pool(name="sD", bufs=2 * ((nwd * mb)
                                                                  // 2 + 4)))
        eD_pool = stk2.enter_context(tc.tile_pool(name="eDp", bufs=6))
        wD = stk2.enter_context(tc.tile_pool(name="wD", bufs=4))
        psD = stk2.enter_context(tc.tile_pool(name="psD", bufs=6,
                                              space="PSUM"))
        psF = stk2.enter_context(tc.tile_pool(name="psF", bufs=2,
                                              space="PSUM"))
        sF = stk2.enter_context(tc.tile_pool(name="sF", bufs=3))

        idx_sb, dstb, nrmb = sb["bip"]

        # group windows for 4 parallel gathers on distinct queues
        ngr = 4
        gsz = (nwd + ngr - 1) // ngr
        groups = [(g0, min(gsz, nwd - g0)) for g0 in range(0, nwd, gsz)]

        gts = []
        for gi, (g0, gn) in enumerate(groups):
            Gt = gD.tile([128, gsz * mb, 2 * H1], BF16, tag=f"Gbip{gi}",
                         name=f"Gbip_{r}_{g0}")
            nc.gpsimd.dma_gather(
                out_ap=Gt[:, :gn * mb, :],
                in_ap=cc_out.ap()[:, :],
                idxs_ap=idx_sb[:, g0 * mb * 8:(g0 + gn) * mb * 8],
                num_idxs=gn * mb * 128,
                num_idxs_reg=gn * mb * 128,
                elem_size=2 * H1, single_packet=False,
                queue_num=gi % 4)
            gts.append(Gt)

        for gi, (g0, gn) in enumerate(groups):
            Gt = gts[gi]
            bipT = wD.tile([H1, gsz * 128], BF16, tag="bipT",
                           name=f"bipT{r}_{g0}")
            for wi in range(gn):
                w = g0 + wi
                accD = psD.tile([H1, W], F32, tag="accD", space="PSUM",
                                name=f"accD{r}_{w}")
                for t in range(mb):
                    col = w * mb + t
                    Sb = sD.tile([128, W], BF16, tag="Sb",
                                 name=f"Sb{r}_{w}_{t}")
                    nc.vector.tensor_scalar(
                        out=Sb[:], in0=iota_sb[:],
                        scalar1=dstb[:, col:col + 1],
                        scalar2=nrmb[:, col:col + 1],
                        op0=mybir.AluOpType.is_equal,
                        op1=mybir.AluOpType.mult)
                    nc.tensor.matmul(out=accD[:],
                                     lhsT=Gt[:, wi * mb + t, :H1],
                                     rhs=Sb[:], start=(t == 0),
                                     stop=(t == mb - 1))
                eD = eD_pool.tile([H1, W], BF16, tag="eD", name=f"eD{r}_{w}")
                nc.scalar.activation(out=eD[:], in_=accD[:],
                                     func=mybir.ActivationFunctionType.Exp)
                zbD = eD_pool.tile([H1, W], BF16, tag="zbD",
                                   name=f"zbD{r}_{w}")
                nc.vector.tensor_scalar(out=zbD[:], in0=accD[:],
                                        scalar1=1.0, scalar2=1.0,
                                        op0=mybir.AluOpType.add,
                                        op1=mybir.AluOpType.max)
                nc.vector.tensor_tensor(out=bipT[:, wi * 128:(wi + 1) * 128],
                                        in0=eD[:], in1=zbD[:],
                                        op=mybir.AluOpType.min)

            # ---- stage F: out^T = W_lin'^T bip' + b'' ----------------------
            fcols = gn * 128
            for j in range((fcols + 511) // 512):
                nt = min(512, fcols - j * 512)
                zF = psF.tile([C, 512], F32, tag="zF", name=f"zF{r}_{g0}_{j}",
                              space="PSUM")
                nc.tensor.matmul(out=zF[:, :nt], lhsT=wlin_sb[:],
                                 rhs=bipT[:, j * 512:j * 512 + nt],
                                 start=True, stop=False)
                nc.tensor.matmul(out=zF[:, :nt], lhsT=blin_sb[:],
                                 rhs=ones_sb[:, :nt], start=False, stop=True)
                oF = sF.tile([C, 512], F32, tag="oF", name=f"oF{r}_{g0}_{j}")
                nc.scalar.copy(out=oF[:, :nt], in_=zF[:, :nt])
                nc.sync.dma_start(
                    out=out_t.ap()[:, g0 * 128 + j * 512:
                                   g0 * 128 + j * 512 + nt],
                    in_=oF[:, :nt])


# ---------------------------------------------------------------------------
# public entry
# ---------------------------------------------------------------------------

def _prepare(inputs, n):
    npc = n // NCORES
    nwd = (npc + 127) // 128

    ei = np.asarray(inputs["edge_index_higher_order"])
    src = ei[0].astype(np.int64)
    dst = ei[1].astype(np.int64)
    ew = np.asarray(inputs["edge_weights_higher_order"]).astype(np.float64)

    bi = np.asarray(inputs["bipartite_edge_index"])
    bsrc = bi[0].astype(np.int64)
    bdst = bi[1].astype(np.int64)

    # degrees over the FULL edge set (self-loop weight 1)
    deg = np.bincount(dst, weights=ew, minlength=n) + 1.0
    dinv = 1.0 / np.sqrt(deg)

    # dead-node pruning: only nodes referenced by a bipartite edge matter
    live = np.zeros(n, bool)
    live[bsrc] = True
    lv = np.nonzero(live)[0]
    nlive = len(lv)

    m = live[dst]
    src_l = src[m]
    dst_l = dst[m]
    norm_l = (dinv[src_l] * ew[m] * dinv[dst_l]).astype(np.float32)
    # fold self-loops in as ordinary edges with norm = dinv^2
    src_all = np.concatenate([src_l, lv])
    dst_all = np.concatenate([dst_l, lv])
    norm_all = np.concatenate([norm_l,
                               (dinv[lv] ** 2).astype(np.float32)])

    # balance live nodes over (core, window) by edge count incl. self-loop
    dcnt = np.bincount(dst_all, minlength=n)[lv]
    total_e = len(src_all)
    nwa = (nlive + 127) // 128 // NCORES + 1
    while True:
        capacity = NCORES * nwa * 2048
        if capacity >= total_e * 1.02 and NCORES * nwa * 128 >= nlive:
            win_of, pos_of, mx = _balance(lv, dcnt.astype(np.float64),
                                          nwa, 2048.0)
            if mx <= 2048:
                break
        nwa += 1
    hcore = np.full(n, -1, np.int64)
    hrow = np.full(n, -1, np.int64)
    hcore[lv] = win_of // nwa
    hrow[lv] = (win_of % nwa) * 128 + pos_of

    ma, bkt_a = _bucket_edges(src_all, hcore[dst_all], hrow[dst_all],
                              norm_all, nwa, pad_idx=-1)

    # ---- bipartite routing: dedup (producer, consumer) rows, fixed block B
    # balance first-order (output) nodes by bipartite in-degree: M_b=1 if
    # every window stays <= 128 edges
    bdeg = np.bincount(bdst, minlength=n).astype(np.float64)
    ocore_w, opos, omx = _balance(np.arange(n), bdeg, nwd, 128.0,
                                  core_cap=npc)
    if omx > 128:
        ocore = np.arange(n) // npc
        orow = np.arange(n) - ocore * npc
    else:
        ocore = ocore_w // nwd
        orow = (ocore_w % nwd) * 128 + opos

    ncons = ocore[bdst]
    nprod = hcore[bsrc]
    srow = hrow[bsrc]          # producer-local msg row of each edge's source
    maxu = 0
    for c in range(NCORES):
        for p in range(NCORES):
            mm = (ncons == c) & (nprod == p)
            maxu = max(maxu, len(np.unique(srow[mm])))
    B = ((maxu + 127) // 128) * 128

    pay_idx = np.zeros((NCORES, NCORES * B), np.int64)
    table_row = np.zeros(len(bsrc), np.int64)
    for c in range(NCORES):
        cm = ncons == c
        for p in range(NCORES):
            mm = cm & (nprod == p)
            uniq, inv = np.unique(srow[mm], return_inverse=True)
            pay_idx[p, c * B:c * B + len(uniq)] = uniq
            table_row[mm] = p * B + inv

    assert NCORES * B <= 32768
    mb, bkt_b = _bucket_edges(table_row, ncons, orow[bdst],
                              np.ones(len(bsrc), np.float32),
                              nwd, pad_idx=0)

    cfg = dict(N=n, NWA=nwa, NWD=nwd, B=B, MA=ma, MB=mb)
    buckets = dict(astream=bkt_a, bip=bkt_b, pay=pay_idx,
                   ocore=ocore, orow=orow)
    return cfg, buckets


def make_in_maps(inputs, cfg, buckets):
    nwa, ma = cfg["NWA"], cfg["MA"]
    x_h = np.asarray(inputs["x_h"], dtype=np.float32).astype(NPBF16)
    x_h = np.ascontiguousarray(x_h)

    W_ho = np.asarray(inputs["W_ho"], np.float32)
    b_ho = np.asarray(inputs["b_ho"], np.float32)
    W_bip = np.asarray(inputs["W_bip1"], np.float32)
    b_bip = np.asarray(inputs["b_bip1"], np.float32)
    W_lin = np.asarray(inputs["W_lin"], np.float32)
    b_lin = np.asarray(inputs["b_lin"], np.float32)

    b_bip_eff = (b_bip - W_bip.sum(axis=0)).reshape(1, H1)
    b_lin_eff = (b_lin - W_lin.sum(axis=0)).reshape(1, C)
    iota = np.broadcast_to(np.arange(W, dtype=np.float32),
                           (128, W)).astype(NPBF16).copy()

    in_maps = []
    for c in range(NCORES):
        src_flat, adst, anrm = buckets["astream"][c]
        gxr = np.zeros((nwa * ma * 128, F), NPBF16)
        emask = src_flat >= 0
        gxr[emask] = x_h[src_flat[emask]]
        gx = np.ascontiguousarray(
            gxr.reshape(nwa * ma, 128, F).transpose(1, 0, 2)
            .reshape(128, nwa * ma * F))
        gi, dl, nr = buckets["bip"][c]
        m = {
            "gx": gx,
            "a_dst": adst,
            "a_nrm": anrm,
            "iota": iota,
            "w_ho": np.ascontiguousarray(W_ho).astype(NPBF16),
            "b_ho": b_ho.reshape(F, 1).astype(np.float32),
            "w_bip": np.ascontiguousarray(W_bip).astype(NPBF16),
            "b_bip": b_bip_eff.astype(NPBF16),
            "w_lin": np.ascontiguousarray(W_lin).astype(NPBF16),
            "b_lin": b_lin_eff.astype(NPBF16),
            "pay_idx": _wrap_idx(buckets["pay"][c]),
            "bip_idx": _wrap_idx(gi),
            "bip_dst": dl,
            "bip_nrm": nr,
        }
        in_maps.append(m)
    return in_maps


def kernel(**inputs):
    x_h = np.asarray(inputs["x_h"])
    n = x_h.shape[0]
    cfg, buckets = _prepare(inputs, n)
    nc = build_nc(cfg)
    in_maps = make_in_maps(inputs, cfg, buckets)
    res = run_bass_kernel_spmd(nc, in_maps, core_ids=list(range(NCORES)))
    arr = np.stack([res.results[c]["outT"] for c in range(NCORES)])
    return np.ascontiguousarray(
        arr[buckets["ocore"], :, buckets["orow"]]).astype(np.float32)
